# revision 30
# baseline (speedup 1.0000x reference)
"""Trainium2 Bass kernel for nn_NeuralNetwork_S (kwta / topk_masking) — v3.

Key design points (vs the original 3-term f32r-split baseline):
- Native fp32 matmuls (HW probe: max rel err 1.8e-7, identical to the
  3-term 12-bit f32r split) -> no hi/lo splits anywhere: half the shipped
  bytes, no host-side rne12, no DVE subtract passes. (bf16/fp16 split
  variants fail the 2e-2 gate: kwta/argmax near-ties amplify operand
  rounding; measured on CPU.)
- Weights ship SHARDED 1/8 per core (18.8MB total instead of 162MB
  replicated) and are AllGather'd on-device into a DRAM bounce buffer as
  one flat 18.8MB collective; per-layer views are rearranged APs into it.
- Host ships raw per-core row slices of state/task (zero-copy views) and
  flat weight-shard views; w^T prep is cached across calls keyed on a
  fingerprint of the weight arrays. ci transpose happens on device
  (PE transpose + DVE copy; GPSIMD cannot read PSUM).
- Biases of the 4 IN-facing layers fold into an augmented K=5 tail matmul
  (task^T rows + ones row) x (w_tail rows + bias row) — free on PE since
  matmul cost is N-driven.
- Software-pipelined emission via woven generators: per group g, phase A
  (ci transpose + l1 + cx chains) and phases B1/B2/B3 (kwta1+l2 / kwta2+l3
  / kwta3+l4) interleave so group g's kwta bisections (ACT/Pool/DVE) hide
  under group g+1's matmul stream (PE stays >80% busy; sim ~1.69ms).
- kwta bisection: per-row dynamic-k threshold found by 12-iter bisection;
  counts on ACT only (sigmoid step w/ 2^100 scale + accum is exact;
  DVE tensor_tensor_reduce and Pool accum_out both break on real HW),
  interval ping-pong arithmetic on Pool, selects on DVE — sized to the
  engines' 4-deep wait stations to avoid queue-head deadlocks.
"""

import sys

_TRN = "/opt/trn_rl_repo"
if _TRN not in sys.path:
    sys.path.insert(0, _TRN)

import numpy as np
import concourse.bass as bass
import concourse.mybir as mybir
import concourse.tile as tile
from concourse import bacc
from concourse.bass_utils import run_bass_kernel_spmd
from concourse.masks import make_identity

P = 128
B = 16384
NCORES = 8
BC = B // NCORES          # 2048 rows per core
BG = 512                  # rows per group
NG = BC // BG             # 4 groups
GT = BG // P              # 4 row-tiles per group
IN = 1028
KIN = 8                   # full 128-row k-chunks of the 1024 state features
HID = 1024
HID2 = 512
HEADS = 128

F32 = mybir.dt.float32
U8 = mybir.dt.uint8
I32 = mybir.dt.int32
U32 = mybir.dt.uint32
BF16 = mybir.dt.bfloat16
AF = mybir.ActivationFunctionType
OP = mybir.AluOpType
AX = mybir.AxisListType

SCALE = float(2.0 ** 100)
ITERS = {1024: 12, 512: 12, 128: 10}
THIRD = 1.0 / 3.0

# layer tables ---------------------------------------------------------------
# IN-layers (read ci): (name, out, form); form 'a' = out-on-partitions,
# 'b' = rows-on-partitions
IN_LAYERS = {"cx11": HID, "cx21": HID2, "cx31": HEADS, "l1": HID}
# hidden layers: name -> (k_in, out)
HID_LAYERS = {"cx12": (HID, HID), "cx22": (HID2, HID2), "cx32": (HEADS, HEADS),
              "l2": (HID, HID2), "l3": (HID2, HEADS), "l4": (HEADS, HEADS)}
W_DIMS = {"l1": (1024, HID), "cx11": (1024, HID), "cx12": (HID, HID),
          "cx21": (1024, HID2), "cx22": (HID2, HID2), "cx31": (1024, HEADS),
          "cx32": (HEADS, HEADS), "l2": (HID, HID2), "l3": (HID2, HEADS),
          "l4": (HEADS, HEADS)}
W_ORDER = ["l1", "cx11", "cx12", "cx21", "cx22", "cx31", "cx32",
           "l2", "l3", "l4"]
W_OFF = {}
_off = 0
for _n in W_ORDER:
    W_OFF[_n] = _off
    _off += W_DIMS[_n][0] * W_DIMS[_n][1]
WTOT = _off          # 4,685,824 floats
WSH = WTOT // NCORES


def build_program():
    nc = bacc.Bacc("TRN2", target_bir_lowering=False, debug=False)
    d = {}

    def din(name, shape, dt=F32):
        d[name] = nc.dram_tensor(name, list(shape), dt, kind="ExternalInput")
        return d[name]

    din("state", [BC, 1024])
    din("task", [BC, 4])
    for name, o in IN_LAYERS.items():
        din(f"{name}_tail", [5, o])
    din("wflat_sh", [WSH])
    for name in ("cx12", "cx22", "cx32", "l2"):
        din(f"{name}_brep", [P, HID_LAYERS[name][1]])
    for name in ("l3", "l4"):
        din(f"{name}_bcol", [P, 1])

    outT = nc.dram_tensor("outT", [P, BC], F32, kind="ExternalOutput")

    with tile.TileContext(nc) as tc:
        _emit(tc, nc, d, outT)
    nc.compile()
    return nc


def _emit(tc, nc, d, outT):
    import contextlib

    ctx = contextlib.ExitStack()
    with ctx:
        big = ctx.enter_context(tc.tile_pool(name="big", bufs=1))
        dbuf = ctx.enter_context(tc.tile_pool(name="dbuf", bufs=1))
        shared = ctx.enter_context(tc.tile_pool(name="shared", bufs=2))
        wts = ctx.enter_context(tc.tile_pool(name="wts", bufs=2))
        cons = ctx.enter_context(tc.tile_pool(name="cons", bufs=1))
        small = ctx.enter_context(tc.tile_pool(name="small", bufs=4))
        scr = ctx.enter_context(tc.tile_pool(name="scr", bufs=1))
        dram = ctx.enter_context(tc.tile_pool(name="dram", bufs=1,
                                               space="DRAM"))
        psb = ctx.enter_context(tc.tile_pool(name="psb", bufs=1, space="PSUM"))
        psa = ctx.enter_context(tc.tile_pool(name="psa", bufs=2, space="PSUM"))
        pst = ctx.enter_context(tc.tile_pool(name="pst", bufs=2, space="PSUM"))

        # constants ----------------------------------------------------------
        ident = cons.tile([P, P], F32, tag="ident")
        make_identity(nc, ident[:])
        negbig = cons.tile([P, 1], F32, tag="negbig")
        nc.vector.memset(negbig[:], -1.0e30)
        iota8 = cons.tile([P, 8], F32, tag="iota8")
        iota8u = small.tile([P, 8], U32, tag="iota8u")
        nc.gpsimd.iota(iota8u[:], pattern=[[1, 8]], base=0, channel_multiplier=0)
        nc.vector.tensor_copy(iota8[:], iota8u[:])
        zbias = cons.tile([P, 1], F32, tag="zbias")
        nc.vector.memset(zbias[:], 0.0)

        # resident weights: tails + breps + bcols -----------------------------
        tails = {}
        for name, o in IN_LAYERS.items():
            t = cons.tile([5, o], F32, tag=f"tail_{name}")
            nc.sync.dma_start(t[:], d[f"{name}_tail"][:])
            tails[name] = t
        breps = {}
        for name in ("cx12", "cx22", "cx32", "l2"):
            t = cons.tile([P, HID_LAYERS[name][1]], F32, tag=f"brep_{name}")
            nc.sync.dma_start(t[:], d[f"{name}_brep"][:])
            breps[name] = t
        bcols = {}
        for name in ("l3", "l4"):
            t = cons.tile([P, 1], F32, tag=f"bcol_{name}")
            nc.sync.dma_start(t[:], d[f"{name}_bcol"][:])
            bcols[name] = t

        state_r = d["state"].rearrange("(n p) f -> p n f", p=P)   # [P,16,1024]
        task_r = d["task"].rearrange("(n p) f -> p n f", p=P)     # [P,16,4]

        # ---- weight all-gather: one flat 18.8MB gather (BW ramps with
        # size; 15us fixed overhead per collective favors a single one).
        gin = dram.tile([WSH], F32, tag="gin")
        gout = dram.tile([WTOT], F32, tag="gout")
        nc.sync.dma_start(gin[:], d["wflat_sh"][:])
        nc.gpsimd.collective_compute(
            "AllGather", mybir.AluOpType.bypass,
            replica_groups=[list(range(NCORES))],
            ins=[gin.opt()], outs=[gout.opt()])
        gathered = {}
        for name in W_ORDER:
            k, o = W_DIMS[name]
            off = W_OFF[name]
            gathered[name] = gout[off:off + k * o].rearrange(
                "(c p o) -> p c o", p=P, o=o)

        def wslab_b(name, k, n0, nw):
            """(b)-form moving slab [P, 1, nw] from wT rows [k*128, +128)."""
            t = wts.tile([P, 1, nw], F32, tag="wb")
            nc.sync.dma_start(t[:], gathered[name][:, k:k + 1, n0:n0 + nw])
            return t

        def wslab_a(name, k0, kc, m0, mw):
            """(a)-form stationary slab [P, kc<=4, mw] (k-chunks k0..k0+kc)."""
            t = wts.tile([P, kc, mw], F32, tag="wa")
            nc.sync.dma_start(t[:], gathered[name][:, k0:k0 + kc, m0:m0 + mw])
            return t

        # ---------------- phase A1: ci transpose, l1, cx1 chain --------------
        def phase_a1(g, st):
            col0 = g * BG
            ciT = shared.tile([P, KIN, BG], F32, tag="big16", name="ciT")
            st["ciT"] = ciT
            taskT = big.tile([5, BG], F32, tag="taskT")
            st["taskT"] = taskT
            tTASK = small.tile([P, GT, 5], F32, tag="tTASK")
            nc.sync.dma_start(tTASK[:, :, 0:4], task_r[:, g * GT:(g + 1) * GT, :])
            nc.gpsimd.memset(tTASK[:, :, 4:5], 1.0)
            yield
            # transpose ci into [feature-part, row] layout
            for t in range(GT):
                sROW = dbuf.tile([P, 1024], F32, tag="sROW")
                nc.sync.dma_start(sROW[:], state_r[:, g * GT + t, :])
                for c0 in (0, 4):
                    ps = pst.tile([P, 4 * P], F32, tag="pst")
                    for c in range(c0, c0 + 4):
                        nc.tensor.transpose(
                            ps[:, (c - c0) * P:(c - c0 + 1) * P],
                            sROW[:, c * P:(c + 1) * P], ident[:])
                    dst = ciT[:, c0:c0 + 4, t * P:(t + 1) * P]
                    src = ps[:].rearrange("p (c q) -> p c q", q=P)
                    nc.vector.tensor_copy(dst, src)
                    yield
                pt = pst.tile([P, 4 * P], F32, tag="pst")
                nc.tensor.transpose(pt[0:5, 0:P], tTASK[:, t, :], ident[:])
                nc.vector.tensor_copy(taskT[0:5, t * P:(t + 1) * P],
                                      pt[0:5, 0:P])
                yield

            # ---- l1 (b): z1 [P, GT, 1024]
            z1 = shared.tile([P, GT, HID], F32, tag="z1", name="z1")
            st["z1"] = z1
            for n0 in range(0, HID, 512):
                ps = psb.tile([P, GT, 512], F32, tag="psb")
                for k in range(KIN):
                    wb = wslab_b("l1", k, n0, 512)
                    for t in range(GT):
                        nc.tensor.matmul(
                            ps[:, t, :], ciT[:, k, t * P:(t + 1) * P],
                            wb[:, 0, :], start=(k == 0), stop=False)
                    yield
                for t in range(GT):
                    nc.tensor.matmul(
                        ps[:, t, :], taskT[0:5, t * P:(t + 1) * P],
                        tails["l1"][0:5, n0:n0 + 512], start=False, stop=True)
                yield
                for t in range(GT):
                    nc.vector.tensor_copy(z1[:, t, n0:n0 + 512], ps[:, t, :])
                yield

            # ---- cx1 chain -> kk0
            yield from cx_chain(g, st, 0)

        # ---------------- phase A2: cx2/cx3 chains ---------------------------
        def phase_a2(g, st):
            yield from cx_chain(g, st, 1)
            yield from cx_chain(g, st, 2)

        CX_DEFS = [("cx11", "cx12", HID, 8), ("cx21", "cx22", HID2, 4),
                   ("cx31", "cx32", HEADS, 1)]

        def cx_chain(g, st, cn):
            ciT = st["ciT"]
            taskT = st["taskT"]
            if True:
                pre, post, hidn, mch = CX_DEFS[cn]
                kc_pre = KIN
                httag = {0: "hx1", 1: "hx2", 2: "hx3"}[cn]
                hT = shared.tile([P, mch, BG], F32, tag=httag, name=f"hT{cn}")
                for m in range(mch):
                    ps = psa.tile([P, BG], F32, tag="psa")
                    for k0 in range(0, kc_pre, 4):
                        wa = wslab_a(pre, k0, 4, m * P, P)
                        for k in range(k0, k0 + 4):
                            nc.tensor.matmul(ps[:], wa[:, k - k0, :],
                                             ciT[:, k, :],
                                             start=(k == 0), stop=False)
                    nc.tensor.matmul(ps[:], tails[pre][0:5, m * P:(m + 1) * P],
                                     taskT[0:5, :], start=False, stop=True)
                    nc.scalar.activation(hT[:, m, :], ps[:], AF.Tanh,
                                         bias=zbias[:], scale=1.0)
                    yield

                # second layer (b) + incremental argmax
                kk = small.tile([P, GT], F32, tag=f"kk{cn}", name="kk")
                st[f"kk{cn}"] = kk
                kin2, out2 = HID_LAYERS[post]
                bestm = small.tile([P, GT], F32, tag="bestm")
                kkA = small.tile([P, GT], F32, tag="kkA")
                n0s = list(range(0, out2, 512))
                for ci_, n0 in enumerate(n0s):
                    nw = min(512, out2)
                    ps = psb.tile([P, GT, 512], F32, tag="psb")
                    for k in range(mch):
                        wb = wslab_b(post, k, n0, nw)
                        for t in range(GT):
                            nc.tensor.matmul(
                                ps[:, t, 0:nw], hT[:, k, t * P:(t + 1) * P],
                                wb[:, 0, :], start=(k == 0), stop=(k == mch - 1))
                        yield
                    m8 = small.tile([P, 8], F32, tag="am8")
                    idx = small.tile([P, 8], U32, tag="aidx")
                    idxf = small.tile([P, 8], F32, tag="aidxf")
                    for t in range(GT):
                        zcx = big.tile([P, 512], F32, tag="zcx", name="zcx")
                        nc.vector.scalar_tensor_tensor(
                            zcx[:, 0:nw], ps[:, t, 0:nw], 1.0,
                            breps[post][:, n0:n0 + nw], op0=OP.mult, op1=OP.add)
                        nc.vector.max(out=m8[:], in_=zcx[:, 0:nw])
                        nc.vector.max_index(idx[:], m8[:], zcx[:, 0:nw])
                        nc.vector.tensor_copy(idxf[:, 0:1], idx[:, 0:1])
                        if ci_ == 0 and len(n0s) == 1:
                            nc.vector.tensor_copy(kk[:, t:t + 1], idxf[:, 0:1])
                        elif ci_ == 0:
                            nc.vector.tensor_copy(kkA[:, t:t + 1], idxf[:, 0:1])
                            nc.vector.tensor_copy(bestm[:, t:t + 1],
                                                  m8[:, 0:1])
                        else:
                            gtu = small.tile([P, 1], U8, tag="agt")
                            nc.vector.tensor_tensor(
                                gtu[:], m8[:, 0:1], bestm[:, t:t + 1],
                                op=OP.is_gt)
                            i2 = small.tile([P, 1], F32, tag="ai2")
                            nc.vector.tensor_scalar(
                                i2[:], idxf[:, 0:1], float(n0), None,
                                op0=OP.add)
                            nc.vector.select(kk[:, t:t + 1], gtu[:], i2[:],
                                             kkA[:, t:t + 1])
                        yield

        # ---------------- kwta bisection ------------------------------------
        def kwta(zg, xg, kk, n):
            I = ITERS[n]
            loA = small.tile([P, GT], F32, tag="kwloA")
            loB = small.tile([P, GT], F32, tag="kwloB")
            hiA = small.tile([P, GT], F32, tag="kwhiA")
            hiB = small.tile([P, GT], F32, tag="kwhiB")
            chA = small.tile([P, GT], F32, tag="kwchA")
            chB = small.tile([P, GT], F32, tag="kwchB")
            cnt = small.tile([P, GT], F32, tag="kwcnt")
            kp1 = small.tile([P, GT], F32, tag="kwkp1")
            msum = small.tile([P, GT], F32, tag="kwmsum")
            mid = small.tile([P, GT], F32, tag="kwmid")
            nbias = small.tile([P, GT], F32, tag="kwnb")
            mn = small.tile([P, GT], F32, tag="kwmn")
            selu = small.tile([P, GT], U8, tag="kwselu")
            trash = scr.tile([P, n], BF16, tag=f"kwA{n}", name="trash")

            nc.gpsimd.tensor_scalar(kp1[:], kk[:], 1.0, None, op0=OP.add)
            nc.gpsimd.memset(chA[:], 0.0)
            for t in range(GT):
                nc.vector.reduce_max(hiA[:, t:t + 1], zg[:, t, :], axis=AX.X)
                nc.vector.tensor_reduce(out=mn[:, t:t + 1], in_=zg[:, t, :],
                                        op=OP.min, axis=AX.X)
            nc.gpsimd.tensor_scalar(loA[:], mn[:], 1.0, None, op0=OP.subtract)
            yield

            lo, hi, ch = loA, hiA, chA
            lon, hin, chn = loB, hiB, chB
            for it in range(I):
                nc.gpsimd.tensor_tensor(msum[:], lo[:], hi[:], op=OP.add)
                nc.gpsimd.tensor_scalar(mid[:], msum[:], 0.5, None,
                                        op0=OP.mult)
                nc.gpsimd.tensor_scalar(nbias[:], mid[:], -SCALE, None,
                                        op0=OP.mult)
                for t in range(GT):
                    nc.scalar.activation(
                        trash[:], zg[:, t, :], AF.Sigmoid,
                        bias=nbias[:, t:t + 1], scale=SCALE,
                        accum_out=cnt[:, t:t + 1])
                nc.vector.tensor_tensor(selu[:], cnt[:], kp1[:], op=OP.is_ge)
                nc.vector.select(lon[:], selu[:], mid[:], lo[:])
                nc.vector.select(hin[:], selu[:], hi[:], mid[:])
                nc.vector.select(chn[:], selu[:], ch[:], cnt[:])
                lo, lon = lon, lo
                hi, hin = hin, hi
                ch, chn = chn, ch
                yield

            chii = small.tile([P, GT], I32, tag="kwchii")
            nc.vector.tensor_scalar(chn[:], ch[:], 0.25, None, op0=OP.subtract)
            nc.vector.tensor_copy(chii[:], chn[:])
            nc.vector.tensor_copy(ch[:], chii[:])
            rm1 = small.tile([P, GT], F32, tag="kwrm1")
            nc.vector.tensor_tensor(rm1[:], kk[:], ch[:], op=OP.subtract)
            yield

            for t in range(GT):
                m1 = scr.tile([P, n], F32, tag=f"kwA{n}", name="m1")
                gu8 = scr.tile([P, n], U8, tag=f"kwgu{n}", name="gu8")
                msk = scr.tile([P, n], F32, tag=f"kwmsk{n}", name="msk")
                nc.gpsimd.tensor_scalar(m1[:], zg[:, t, :], lo[:, t:t + 1],
                                        None, op0=OP.max)
                nc.vector.tensor_scalar(gu8[:], zg[:, t, :], hi[:, t:t + 1],
                                        None, op0=OP.is_gt)
                nc.vector.select(msk[:], gu8[:], negbig[:].to_broadcast([P, n]),
                                 m1[:])
                m8 = small.tile([P, 8], F32, tag="kwm8")
                nc.vector.max(out=m8[:], in_=msk[:])
                eq = small.tile([P, 8], F32, tag="kweq")
                nc.vector.tensor_scalar(eq[:], iota8[:], rm1[:, t:t + 1],
                                        None, op0=OP.is_equal)
                pr = small.tile([P, 8], F32, tag="kwpr")
                nc.vector.tensor_tensor(pr[:], eq[:], m8[:], op=OP.mult)
                u = small.tile([P, 1], F32, tag="kwu")
                nc.vector.reduce_sum(u[:], pr[:], axis=AX.X)
                yield
                geu = scr.tile([P, n], U8, tag=f"kwgu{n}", name="geu")
                nc.vector.tensor_scalar(geu[:], zg[:, t, :], u[:], None,
                                        op0=OP.is_gt)
                zth = scr.tile([P, n], F32, tag=f"kwA{n}", name="zth")
                nc.gpsimd.tensor_scalar(zth[:], zg[:, t, :], THIRD, None,
                                        op0=OP.mult)
                nc.vector.select(xg[:, t, :], geu[:], zg[:, t, :], zth[:])
                yield

        # transpose [P, GT, n] -> xT [P, n//P, BG]
        def transpose_x(xg, xT, n):
            nch = n // P
            for t in range(GT):
                for c0 in range(0, nch, 4):
                    cw = min(4, nch - c0)
                    ps = pst.tile([P, 4 * P], F32, tag="pst")
                    for c in range(c0, c0 + cw):
                        nc.tensor.transpose(
                            ps[:, (c - c0) * P:(c - c0 + 1) * P],
                            xg[:, t, c * P:(c + 1) * P], ident[:])
                    dst = xT[:, c0:c0 + cw, t * P:(t + 1) * P]
                    src = ps[:, 0:cw * P].rearrange("p (c q) -> p c q", q=P)
                    nc.vector.tensor_copy(dst, src)
                    yield

        # ---------------- phase B1: kwta1, x1T, l2 ---------------------------
        def phase_b1(g, st):
            x1 = shared.tile([P, GT, HID], F32, tag="big16", name="x1")
            yield from kwta(st["z1"], x1, st["kk0"], HID)
            x1T = shared.tile([P, HID // P, BG], F32, tag="hx1", name="x1T")
            yield from transpose_x(x1, x1T, HID)
            z2 = shared.tile([P, GT, HID2], F32, tag="z2")
            st["z2"] = z2
            ps = psb.tile([P, GT, 512], F32, tag="psb")
            for k in range(HID // P):
                wb = wslab_b("l2", k, 0, HID2)
                for t in range(GT):
                    nc.tensor.matmul(
                        ps[:, t, :], x1T[:, k, t * P:(t + 1) * P],
                        wb[:, 0, :], start=(k == 0), stop=(k == HID // P - 1))
                yield
            for t in range(GT):
                nc.vector.scalar_tensor_tensor(
                    z2[:, t, :], ps[:, t, :], 1.0, breps["l2"][:],
                    op0=OP.mult, op1=OP.add)
            yield

        # ---------------- phase B2: kwta2, x2T, l3 ---------------------------
        def phase_b2(g, st):
            x2 = big.tile([P, GT, HID2], F32, tag="x2")
            yield from kwta(st["z2"], x2, st["kk1"], HID2)
            x2T = shared.tile([P, HID2 // P, BG], F32, tag="hx2", name="x2T")
            yield from transpose_x(x2, x2T, HID2)
            ps3 = psa.tile([P, BG], F32, tag="psa")
            wa = wslab_a("l3", 0, HID2 // P, 0, P)
            for k in range(HID2 // P):
                nc.tensor.matmul(ps3[:], wa[:, k, :], x2T[:, k, :],
                                 start=(k == 0), stop=(k == HID2 // P - 1))
            z3T = big.tile([P, BG], F32, tag="zot", name="z3T")
            nc.vector.scalar_tensor_tensor(
                z3T[:], ps3[:], 1.0, bcols["l3"][:].to_broadcast([P, BG]),
                op0=OP.mult, op1=OP.add)
            yield
            z3 = shared.tile([P, GT, HEADS], F32, tag="z3")
            st["z3"] = z3
            for t in range(GT):
                pt = pst.tile([P, 4 * P], F32, tag="pst")
                nc.tensor.transpose(pt[:, 0:P], z3T[:, t * P:(t + 1) * P],
                                    ident[:])
                nc.vector.tensor_copy(z3[:, t, :], pt[:, 0:P])
            yield

        # ---------------- phase B3: kwta3, x3T, l4, out ----------------------
        def phase_b3(g, st):
            col0 = g * BG
            x3 = big.tile([P, GT, HEADS], F32, tag="x3")
            yield from kwta(st["z3"], x3, st["kk2"], HEADS)
            x3T = shared.tile([P, 1, BG], F32, tag="hx3", name="x3T")
            yield from transpose_x(x3, x3T, HEADS)
            ps4 = psa.tile([P, BG], F32, tag="psa")
            wa = wslab_a("l4", 0, 1, 0, P)
            nc.tensor.matmul(ps4[:], wa[:, 0, :], x3T[:, 0, :],
                             start=True, stop=True)
            og = big.tile([P, BG], F32, tag="zot", name="og")
            nc.vector.scalar_tensor_tensor(
                og[:], ps4[:], 1.0, bcols["l4"][:].to_broadcast([P, BG]),
                op0=OP.mult, op1=OP.add)
            nc.sync.dma_start(outT[:, col0:col0 + BG], og[:])
            yield

        # ---------------- weave ------------------------------------------
        sts = [dict() for _ in range(NG)]

        def weave(gens):
            active = list(gens)
            while active:
                keep = []
                for it in active:
                    try:
                        next(it)
                        keep.append(it)
                    except StopIteration:
                        pass
                active = keep

        def phase_a(g, st):
            yield from phase_a1(g, st)
            yield from phase_a2(g, st)

        def seq(*gens):
            for gi in gens:
                yield from gi

        slots = [
            [phase_a(0, sts[0])],
            [phase_a(1, sts[1])],
            [phase_a(2, sts[2]), phase_b1(0, sts[0])],
            [phase_a(3, sts[3]), phase_b2(0, sts[0]), phase_b1(1, sts[1])],
            [phase_b3(0, sts[0]), phase_b2(1, sts[1]),
             seq(phase_b1(2, sts[2]), phase_b1(3, sts[3]))],
            [phase_b3(1, sts[1]),
             seq(phase_b2(2, sts[2]), phase_b2(3, sts[3]))],
            [seq(phase_b3(2, sts[2]), phase_b3(3, sts[3]))],
        ]
        for s in slots:
            weave(s)


# ----------------------------------------------------------------------------
# host wrapper
# ----------------------------------------------------------------------------

_CACHE = {}


def _get_program():
    if "nc" not in _CACHE:
        _CACHE["nc"] = build_program()
    return _CACHE["nc"]


def _fingerprint(arrs):
    out = []
    for a in arrs:
        out.append((id(a), a.shape, a.dtype.str,
                    float(a.flat[0]), float(a.flat[-1])))
    return tuple(out)


def _prep_weights(ws):
    """ws: dict name -> (w, b). Returns the replicated input map (cached)."""
    arrs = [a for pair in ws.values() for a in pair]
    key = _fingerprint(arrs)
    hit = _CACHE.get("wkey")
    if hit == key:
        return _CACHE["wmap"]
    m = {}
    shards = {}
    for name, (w, b) in ws.items():
        w = np.asarray(w, dtype=np.float32)
        b = np.asarray(b, dtype=np.float32)
        if name in IN_LAYERS:
            wT = np.ascontiguousarray(w[:, :1024].T)
            m[f"{name}_tail"] = np.ascontiguousarray(
                np.vstack([w[:, 1024:1028].T, b[None, :]]))
        else:
            wT = np.ascontiguousarray(w.T)
            if name in ("l3", "l4"):
                m[f"{name}_bcol"] = np.ascontiguousarray(
                    np.broadcast_to(b[:, None], (P, 1)))
            else:
                m[f"{name}_brep"] = np.ascontiguousarray(
                    np.broadcast_to(b[None, :], (P, b.shape[0])))
        shards[name] = wT
    wflat = np.concatenate([shards[n].reshape(-1) for n in W_ORDER])
    wsh = [wflat[c * WSH:(c + 1) * WSH] for c in range(NCORES)]
    _CACHE["wkey"] = key
    _CACHE["wmap"] = (m, {"wflat_sh": wsh})
    return m, {"wflat_sh": wsh}


def kernel(**inputs):
    _trace = bool(inputs.pop("_trace", False))
    nc = _get_program()
    state = np.asarray(inputs["state"], dtype=np.float32)
    task = np.asarray(inputs["task_indicator"], dtype=np.float32)
    ws = {n: (inputs[f"{n}_w"], inputs[f"{n}_b"])
          for n in list(IN_LAYERS) + list(HID_LAYERS)}
    common, shards = _prep_weights(ws)
    in_maps = []
    for c in range(NCORES):
        m = dict(common)
        m["state"] = state[c * BC:(c + 1) * BC]
        m["task"] = task[c * BC:(c + 1) * BC]
        for sk, sv in shards.items():
            m[sk] = sv[c]
        in_maps.append(m)
    res = run_bass_kernel_spmd(nc, in_maps, core_ids=list(range(NCORES)),
                               trace=_trace)
    kernel.last_exec_time_ns = res.exec_time_ns
    out = np.concatenate([r["outT"].T for r in res.results], axis=0)
    return np.ascontiguousarray(out, dtype=np.float32)


kernel.last_exec_time_ns = None


# revision 32
# speedup vs baseline: 1.1446x; 1.1446x over previous
"""Trainium2 Bass kernel for nn_NeuralNetwork_S (kwta / topk_masking) — v3.

Key design points (vs the original 3-term f32r-split baseline):
- Native fp32 matmuls (HW probe: max rel err 1.8e-7, identical to the
  3-term 12-bit f32r split) -> no hi/lo splits anywhere: half the shipped
  bytes, no host-side rne12, no DVE subtract passes. (bf16/fp16 split
  variants fail the 2e-2 gate: kwta/argmax near-ties amplify operand
  rounding; measured on CPU.)
- Weights ship SHARDED 1/8 per core (18.8MB total instead of 162MB
  replicated) and are AllGather'd on-device into a DRAM bounce buffer as
  one flat 18.8MB collective; per-layer views are rearranged APs into it.
- Host ships raw per-core row slices of state/task (zero-copy views) and
  flat weight-shard views; w^T prep is cached across calls keyed on a
  fingerprint of the weight arrays. ci transpose happens on device
  (PE transpose + DVE copy; GPSIMD cannot read PSUM).
- Biases of the 4 IN-facing layers fold into an augmented K=5 tail matmul
  (task^T rows + ones row) x (w_tail rows + bias row) — free on PE since
  matmul cost is N-driven.
- Software-pipelined emission via woven generators: per group g, phase A
  (ci transpose + l1 + cx chains) and phases B1/B2/B3 (kwta1+l2 / kwta2+l3
  / kwta3+l4) interleave so group g's kwta bisections (ACT/Pool/DVE) hide
  under group g+1's matmul stream (PE stays >80% busy; sim ~1.69ms).
- kwta bisection: per-row dynamic-k threshold found by 12-iter bisection;
  counts on ACT only (sigmoid step w/ 2^100 scale + accum is exact;
  DVE tensor_tensor_reduce and Pool accum_out both break on real HW),
  interval ping-pong arithmetic on Pool, selects on DVE — sized to the
  engines' 4-deep wait stations to avoid queue-head deadlocks.
"""

import sys

_TRN = "/opt/trn_rl_repo"
if _TRN not in sys.path:
    sys.path.insert(0, _TRN)

import numpy as np
import concourse.bass as bass
import concourse.mybir as mybir
import concourse.tile as tile
from concourse import bacc
from concourse.bass_utils import run_bass_kernel_spmd
from concourse.masks import make_identity

P = 128
B = 16384
NCORES = 8
BC = B // NCORES          # 2048 rows per core
BG = 512                  # rows per group
NG = BC // BG             # 4 groups
GT = BG // P              # 4 row-tiles per group
IN = 1028
KIN = 8                   # full 128-row k-chunks of the 1024 state features
HID = 1024
HID2 = 512
HEADS = 128

F32 = mybir.dt.float32
U8 = mybir.dt.uint8
I32 = mybir.dt.int32
U32 = mybir.dt.uint32
BF16 = mybir.dt.bfloat16
AF = mybir.ActivationFunctionType
OP = mybir.AluOpType
AX = mybir.AxisListType

SCALE = float(2.0 ** 100)
ITERS = {1024: 12, 512: 12, 128: 10}
THIRD = 1.0 / 3.0

# layer tables ---------------------------------------------------------------
# IN-layers (read ci): (name, out, form); form 'a' = out-on-partitions,
# 'b' = rows-on-partitions
IN_LAYERS = {"cx11": HID, "cx21": HID2, "cx31": HEADS, "l1": HID}
# hidden layers: name -> (k_in, out)
HID_LAYERS = {"cx12": (HID, HID), "cx22": (HID2, HID2), "cx32": (HEADS, HEADS),
              "l2": (HID, HID2), "l3": (HID2, HEADS), "l4": (HEADS, HEADS)}
W_DIMS = {"l1": (1024, HID), "cx11": (1024, HID), "cx12": (HID, HID),
          "cx21": (1024, HID2), "cx22": (HID2, HID2), "cx31": (1024, HEADS),
          "cx32": (HEADS, HEADS), "l2": (HID, HID2), "l3": (HID2, HEADS),
          "l4": (HEADS, HEADS)}
W_ORDER = ["l1", "cx11", "cx12", "cx21", "cx22", "cx31", "cx32",
           "l2", "l3", "l4"]
W_OFF = {}
_off = 0
for _n in W_ORDER:
    W_OFF[_n] = _off
    _off += W_DIMS[_n][0] * W_DIMS[_n][1]
WTOT = _off          # 4,685,824 floats
WSH = WTOT // NCORES


def build_program():
    nc = bacc.Bacc("TRN2", target_bir_lowering=False, debug=False)
    d = {}

    def din(name, shape, dt=F32):
        d[name] = nc.dram_tensor(name, list(shape), dt, kind="ExternalInput")
        return d[name]

    din("state", [BC, 1024])
    din("task", [BC, 4])
    for name, o in IN_LAYERS.items():
        din(f"{name}_tail", [5, o])
    din("wflat_sh", [WSH])
    din("brows", [1, 2176])   # cx12(1024) | cx22(512) | cx32(128) | l2(512)
    for name in ("l3", "l4"):
        din(f"{name}_bcol", [P, 1])

    outT = nc.dram_tensor("outT", [P, BC], F32, kind="ExternalOutput")

    with tile.TileContext(nc) as tc:
        _emit(tc, nc, d, outT)
    nc.compile()
    return nc


def _emit(tc, nc, d, outT):
    import contextlib

    ctx = contextlib.ExitStack()
    with ctx:
        big = ctx.enter_context(tc.tile_pool(name="big", bufs=1))
        dbuf = ctx.enter_context(tc.tile_pool(name="dbuf", bufs=1))
        shared = ctx.enter_context(tc.tile_pool(name="shared", bufs=2))
        wts = ctx.enter_context(tc.tile_pool(name="wts", bufs=2))
        cons = ctx.enter_context(tc.tile_pool(name="cons", bufs=1))
        small = ctx.enter_context(tc.tile_pool(name="small", bufs=4))
        scr = ctx.enter_context(tc.tile_pool(name="scr", bufs=1))
        dram = ctx.enter_context(tc.tile_pool(name="dram", bufs=1,
                                               space="DRAM"))
        psb = ctx.enter_context(tc.tile_pool(name="psb", bufs=1, space="PSUM"))
        psa = ctx.enter_context(tc.tile_pool(name="psa", bufs=2, space="PSUM"))
        pst = ctx.enter_context(tc.tile_pool(name="pst", bufs=2, space="PSUM"))

        # constants ----------------------------------------------------------
        ident = cons.tile([P, P], F32, tag="ident")
        make_identity(nc, ident[:])
        negbig = cons.tile([P, 1], F32, tag="negbig")
        nc.vector.memset(negbig[:], -1.0e30)
        iota8 = cons.tile([P, 8], F32, tag="iota8")
        iota8u = small.tile([P, 8], U32, tag="iota8u")
        nc.gpsimd.iota(iota8u[:], pattern=[[1, 8]], base=0, channel_multiplier=0)
        nc.vector.tensor_copy(iota8[:], iota8u[:])
        zbias = cons.tile([P, 1], F32, tag="zbias")
        nc.vector.memset(zbias[:], 0.0)

        # resident weights: tails + breps + bcols -----------------------------
        tails = {}
        for name, o in IN_LAYERS.items():
            t = cons.tile([5, o], F32, tag=f"tail_{name}")
            nc.sync.dma_start(t[:], d[f"{name}_tail"][:])
            tails[name] = t
        brow = shared.tile([1, 2176], F32, tag="big16", name="brow")
        nc.sync.dma_start(brow[:], d["brows"][:])
        ones1 = small.tile([1, P], F32, tag="ones1")
        nc.vector.memset(ones1[:], 1.0)
        breps = {}
        _boff = 0
        for name in ("cx12", "cx22", "cx32", "l2"):
            o = HID_LAYERS[name][1]
            t = cons.tile([P, o], F32, tag=f"brep_{name}")
            for c0 in range(0, o, 512):
                cw = min(512, o - c0)
                psB = psa.tile([P, BG], F32, tag="psa", name="psB")
                nc.tensor.matmul(psB[:, 0:cw], ones1[0:1, :],
                                 brow[0:1, _boff + c0:_boff + c0 + cw],
                                 start=True, stop=True)
                nc.vector.tensor_copy(t[:, c0:c0 + cw], psB[:, 0:cw])
            breps[name] = t
            _boff += o
        bcols = {}
        for name in ("l3", "l4"):
            t = cons.tile([P, 1], F32, tag=f"bcol_{name}")
            nc.sync.dma_start(t[:], d[f"{name}_bcol"][:])
            bcols[name] = t

        state_r = d["state"].rearrange("(n p) f -> p n f", p=P)   # [P,16,1024]
        task_r = d["task"].rearrange("(n p) f -> p n f", p=P)     # [P,16,4]

        # ---- weight all-gather: one flat 18.8MB gather (BW ramps with
        # size; 15us fixed overhead per collective favors a single one).
        gin = dram.tile([WSH], F32, tag="gin")
        gout = dram.tile([WTOT], F32, tag="gout")
        nc.sync.dma_start(gin[:], d["wflat_sh"][:])
        nc.gpsimd.collective_compute(
            "AllGather", mybir.AluOpType.bypass,
            replica_groups=[list(range(NCORES))],
            ins=[gin.opt()], outs=[gout.opt()])
        gathered = {}
        for name in W_ORDER:
            k, o = W_DIMS[name]
            off = W_OFF[name]
            gathered[name] = gout[off:off + k * o].rearrange(
                "(c p o) -> p c o", p=P, o=o)

        def wslab_b(name, k, n0, nw):
            """(b)-form moving slab [P, 1, nw] from wT rows [k*128, +128)."""
            t = wts.tile([P, 1, nw], F32, tag="wb")
            nc.sync.dma_start(t[:], gathered[name][:, k:k + 1, n0:n0 + nw])
            return t

        def wslab_a(name, k0, kc, m0, mw):
            """(a)-form stationary slab [P, kc<=4, mw] (k-chunks k0..k0+kc)."""
            t = wts.tile([P, kc, mw], F32, tag="wa")
            nc.sync.dma_start(t[:], gathered[name][:, k0:k0 + kc, m0:m0 + mw])
            return t

        # ---------------- phase A1: ci transpose, l1, cx1 chain --------------
        def phase_a1(g, st):
            col0 = g * BG
            ciT = shared.tile([P, KIN, BG], F32, tag="big16", name="ciT")
            st["ciT"] = ciT
            taskT = big.tile([5, BG], F32, tag="taskT")
            st["taskT"] = taskT
            tTASK = small.tile([P, GT, 5], F32, tag="tTASK")
            nc.sync.dma_start(tTASK[:, :, 0:4], task_r[:, g * GT:(g + 1) * GT, :])
            nc.gpsimd.memset(tTASK[:, :, 4:5], 1.0)
            yield
            # transpose ci into [feature-part, row] layout
            for t in range(GT):
                sROW = dbuf.tile([P, 1024], F32, tag="sROW")
                nc.sync.dma_start(sROW[:], state_r[:, g * GT + t, :])
                for c0 in (0, 4):
                    ps = pst.tile([P, 4 * P], F32, tag="pst")
                    for c in range(c0, c0 + 4):
                        nc.tensor.transpose(
                            ps[:, (c - c0) * P:(c - c0 + 1) * P],
                            sROW[:, c * P:(c + 1) * P], ident[:])
                    dst = ciT[:, c0:c0 + 4, t * P:(t + 1) * P]
                    src = ps[:].rearrange("p (c q) -> p c q", q=P)
                    nc.vector.tensor_copy(dst, src)
                    yield
                pt = pst.tile([P, 4 * P], F32, tag="pst")
                nc.tensor.transpose(pt[0:5, 0:P], tTASK[:, t, :], ident[:])
                nc.vector.tensor_copy(taskT[0:5, t * P:(t + 1) * P],
                                      pt[0:5, 0:P])
                yield

            # ---- l1 (b): z1 [P, GT, 1024]
            z1 = shared.tile([P, GT, HID], F32, tag="z1", name="z1")
            st["z1"] = z1
            for n0 in range(0, HID, 512):
                ps = psb.tile([P, GT, 512], F32, tag="psb")
                for k in range(KIN):
                    wb = wslab_b("l1", k, n0, 512)
                    for t in range(GT):
                        nc.tensor.matmul(
                            ps[:, t, :], ciT[:, k, t * P:(t + 1) * P],
                            wb[:, 0, :], start=(k == 0), stop=False)
                    yield
                for t in range(GT):
                    nc.tensor.matmul(
                        ps[:, t, :], taskT[0:5, t * P:(t + 1) * P],
                        tails["l1"][0:5, n0:n0 + 512], start=False, stop=True)
                yield
                for t in range(GT):
                    nc.vector.tensor_copy(z1[:, t, n0:n0 + 512], ps[:, t, :])
                yield

            # ---- cx1 chain -> kk0
            yield from cx_chain(g, st, 0)

        # ---------------- phase A2: cx2/cx3 chains ---------------------------
        def phase_a2(g, st):
            yield from cx_chain(g, st, 1)
            yield from cx_chain(g, st, 2)

        CX_DEFS = [("cx11", "cx12", HID, 8), ("cx21", "cx22", HID2, 4),
                   ("cx31", "cx32", HEADS, 1)]

        def cx_chain(g, st, cn):
            ciT = st["ciT"]
            taskT = st["taskT"]
            if True:
                pre, post, hidn, mch = CX_DEFS[cn]
                kc_pre = KIN
                httag = {0: "hx1", 1: "hx2", 2: "hx3"}[cn]
                hT = shared.tile([P, mch, BG], F32, tag=httag, name=f"hT{cn}")
                for m in range(mch):
                    ps = psa.tile([P, BG], F32, tag="psa")
                    for k0 in range(0, kc_pre, 4):
                        wa = wslab_a(pre, k0, 4, m * P, P)
                        for k in range(k0, k0 + 4):
                            nc.tensor.matmul(ps[:], wa[:, k - k0, :],
                                             ciT[:, k, :],
                                             start=(k == 0), stop=False)
                    nc.tensor.matmul(ps[:], tails[pre][0:5, m * P:(m + 1) * P],
                                     taskT[0:5, :], start=False, stop=True)
                    nc.scalar.activation(hT[:, m, :], ps[:], AF.Tanh,
                                         bias=zbias[:], scale=1.0)
                    yield

                # second layer (b) + incremental argmax
                kk = small.tile([P, GT], F32, tag=f"kk{cn}", name="kk")
                st[f"kk{cn}"] = kk
                kin2, out2 = HID_LAYERS[post]
                bestm = small.tile([P, GT], F32, tag="bestm")
                kkA = small.tile([P, GT], F32, tag="kkA")
                n0s = list(range(0, out2, 512))
                for ci_, n0 in enumerate(n0s):
                    nw = min(512, out2)
                    ps = psb.tile([P, GT, 512], F32, tag="psb")
                    for k in range(mch):
                        wb = wslab_b(post, k, n0, nw)
                        for t in range(GT):
                            nc.tensor.matmul(
                                ps[:, t, 0:nw], hT[:, k, t * P:(t + 1) * P],
                                wb[:, 0, :], start=(k == 0), stop=(k == mch - 1))
                        yield
                    m8 = small.tile([P, 8], F32, tag="am8")
                    idx = small.tile([P, 8], U32, tag="aidx")
                    idxf = small.tile([P, 8], F32, tag="aidxf")
                    for t in range(GT):
                        zcx = big.tile([P, 512], F32, tag="zcx", name="zcx")
                        nc.vector.scalar_tensor_tensor(
                            zcx[:, 0:nw], ps[:, t, 0:nw], 1.0,
                            breps[post][:, n0:n0 + nw], op0=OP.mult, op1=OP.add)
                        nc.vector.max(out=m8[:], in_=zcx[:, 0:nw])
                        nc.vector.max_index(idx[:], m8[:], zcx[:, 0:nw])
                        nc.vector.tensor_copy(idxf[:, 0:1], idx[:, 0:1])
                        if ci_ == 0 and len(n0s) == 1:
                            nc.vector.tensor_copy(kk[:, t:t + 1], idxf[:, 0:1])
                        elif ci_ == 0:
                            nc.vector.tensor_copy(kkA[:, t:t + 1], idxf[:, 0:1])
                            nc.vector.tensor_copy(bestm[:, t:t + 1],
                                                  m8[:, 0:1])
                        else:
                            gtu = small.tile([P, 1], U8, tag="agt")
                            nc.vector.tensor_tensor(
                                gtu[:], m8[:, 0:1], bestm[:, t:t + 1],
                                op=OP.is_gt)
                            i2 = small.tile([P, 1], F32, tag="ai2")
                            nc.vector.tensor_scalar(
                                i2[:], idxf[:, 0:1], float(n0), None,
                                op0=OP.add)
                            nc.vector.select(kk[:, t:t + 1], gtu[:], i2[:],
                                             kkA[:, t:t + 1])
                        yield

        # ---------------- kwta bisection ------------------------------------
        def kwta(zg, xg, kk, n):
            I = ITERS[n]
            loA = small.tile([P, GT], F32, tag="kwloA")
            loB = small.tile([P, GT], F32, tag="kwloB")
            hiA = small.tile([P, GT], F32, tag="kwhiA")
            hiB = small.tile([P, GT], F32, tag="kwhiB")
            chA = small.tile([P, GT], F32, tag="kwchA")
            chB = small.tile([P, GT], F32, tag="kwchB")
            cnt = small.tile([P, GT], F32, tag="kwcnt")
            kp1 = small.tile([P, GT], F32, tag="kwkp1")
            msum = small.tile([P, GT], F32, tag="kwmsum")
            mid = small.tile([P, GT], F32, tag="kwmid")
            nbias = small.tile([P, GT], F32, tag="kwnb")
            mn = small.tile([P, GT], F32, tag="kwmn")
            selu = small.tile([P, GT], U8, tag="kwselu")
            trash = scr.tile([P, n], BF16, tag=f"kwA{n}", name="trash")

            nc.gpsimd.tensor_scalar(kp1[:], kk[:], 1.0, None, op0=OP.add)
            nc.gpsimd.memset(chA[:], 0.0)
            for t in range(GT):
                nc.vector.reduce_max(hiA[:, t:t + 1], zg[:, t, :], axis=AX.X)
                nc.vector.tensor_reduce(out=mn[:, t:t + 1], in_=zg[:, t, :],
                                        op=OP.min, axis=AX.X)
            nc.gpsimd.tensor_scalar(loA[:], mn[:], 1.0, None, op0=OP.subtract)
            yield

            lo, hi, ch = loA, hiA, chA
            lon, hin, chn = loB, hiB, chB
            for it in range(I):
                nc.gpsimd.tensor_tensor(msum[:], lo[:], hi[:], op=OP.add)
                nc.gpsimd.tensor_scalar(mid[:], msum[:], 0.5, None,
                                        op0=OP.mult)
                nc.gpsimd.tensor_scalar(nbias[:], mid[:], -SCALE, None,
                                        op0=OP.mult)
                for t in range(GT):
                    nc.scalar.activation(
                        trash[:], zg[:, t, :], AF.Sigmoid,
                        bias=nbias[:, t:t + 1], scale=SCALE,
                        accum_out=cnt[:, t:t + 1])
                nc.vector.tensor_tensor(selu[:], cnt[:], kp1[:], op=OP.is_ge)
                nc.vector.select(lon[:], selu[:], mid[:], lo[:])
                nc.vector.select(hin[:], selu[:], hi[:], mid[:])
                nc.vector.select(chn[:], selu[:], ch[:], cnt[:])
                lo, lon = lon, lo
                hi, hin = hin, hi
                ch, chn = chn, ch
                yield

            chii = small.tile([P, GT], I32, tag="kwchii")
            nc.vector.tensor_scalar(chn[:], ch[:], 0.25, None, op0=OP.subtract)
            nc.vector.tensor_copy(chii[:], chn[:])
            nc.vector.tensor_copy(ch[:], chii[:])
            rm1 = small.tile([P, GT], F32, tag="kwrm1")
            nc.vector.tensor_tensor(rm1[:], kk[:], ch[:], op=OP.subtract)
            yield

            for t in range(GT):
                m1 = scr.tile([P, n], F32, tag=f"kwA{n}", name="m1")
                gu8 = scr.tile([P, n], U8, tag=f"kwgu{n}", name="gu8")
                msk = scr.tile([P, n], F32, tag=f"kwmsk{n}", name="msk")
                nc.gpsimd.tensor_scalar(m1[:], zg[:, t, :], lo[:, t:t + 1],
                                        None, op0=OP.max)
                nc.vector.tensor_scalar(gu8[:], zg[:, t, :], hi[:, t:t + 1],
                                        None, op0=OP.is_gt)
                nc.vector.select(msk[:], gu8[:], negbig[:].to_broadcast([P, n]),
                                 m1[:])
                m8 = small.tile([P, 8], F32, tag="kwm8")
                nc.vector.max(out=m8[:], in_=msk[:])
                eq = small.tile([P, 8], F32, tag="kweq")
                nc.vector.tensor_scalar(eq[:], iota8[:], rm1[:, t:t + 1],
                                        None, op0=OP.is_equal)
                pr = small.tile([P, 8], F32, tag="kwpr")
                nc.vector.tensor_tensor(pr[:], eq[:], m8[:], op=OP.mult)
                u = small.tile([P, 1], F32, tag="kwu")
                nc.vector.reduce_sum(u[:], pr[:], axis=AX.X)
                yield
                geu = scr.tile([P, n], U8, tag=f"kwgu{n}", name="geu")
                nc.vector.tensor_scalar(geu[:], zg[:, t, :], u[:], None,
                                        op0=OP.is_gt)
                zth = scr.tile([P, n], F32, tag=f"kwA{n}", name="zth")
                nc.gpsimd.tensor_scalar(zth[:], zg[:, t, :], THIRD, None,
                                        op0=OP.mult)
                nc.vector.select(xg[:, t, :], geu[:], zg[:, t, :], zth[:])
                yield

        # transpose [P, GT, n] -> xT [P, n//P, BG]
        def transpose_x(xg, xT, n):
            nch = n // P
            for t in range(GT):
                for c0 in range(0, nch, 4):
                    cw = min(4, nch - c0)
                    ps = pst.tile([P, 4 * P], F32, tag="pst")
                    for c in range(c0, c0 + cw):
                        nc.tensor.transpose(
                            ps[:, (c - c0) * P:(c - c0 + 1) * P],
                            xg[:, t, c * P:(c + 1) * P], ident[:])
                    dst = xT[:, c0:c0 + cw, t * P:(t + 1) * P]
                    src = ps[:, 0:cw * P].rearrange("p (c q) -> p c q", q=P)
                    nc.vector.tensor_copy(dst, src)
                    yield

        # ---------------- phase B1: kwta1, x1T, l2 ---------------------------
        def phase_b1(g, st):
            x1 = shared.tile([P, GT, HID], F32, tag="big16", name="x1")
            yield from kwta(st["z1"], x1, st["kk0"], HID)
            x1T = shared.tile([P, HID // P, BG], F32, tag="hx1", name="x1T")
            yield from transpose_x(x1, x1T, HID)
            z2 = shared.tile([P, GT, HID2], F32, tag="z2")
            st["z2"] = z2
            ps = psb.tile([P, GT, 512], F32, tag="psb")
            for k in range(HID // P):
                wb = wslab_b("l2", k, 0, HID2)
                for t in range(GT):
                    nc.tensor.matmul(
                        ps[:, t, :], x1T[:, k, t * P:(t + 1) * P],
                        wb[:, 0, :], start=(k == 0), stop=(k == HID // P - 1))
                yield
            for t in range(GT):
                nc.vector.scalar_tensor_tensor(
                    z2[:, t, :], ps[:, t, :], 1.0, breps["l2"][:],
                    op0=OP.mult, op1=OP.add)
            yield

        # ---------------- phase B2: kwta2, x2T, l3 ---------------------------
        def phase_b2(g, st):
            x2 = big.tile([P, GT, HID2], F32, tag="x2")
            yield from kwta(st["z2"], x2, st["kk1"], HID2)
            x2T = shared.tile([P, HID2 // P, BG], F32, tag="hx2", name="x2T")
            yield from transpose_x(x2, x2T, HID2)
            ps3 = psa.tile([P, BG], F32, tag="psa")
            wa = wslab_a("l3", 0, HID2 // P, 0, P)
            for k in range(HID2 // P):
                nc.tensor.matmul(ps3[:], wa[:, k, :], x2T[:, k, :],
                                 start=(k == 0), stop=(k == HID2 // P - 1))
            z3T = big.tile([P, BG], F32, tag="zot", name="z3T")
            nc.vector.scalar_tensor_tensor(
                z3T[:], ps3[:], 1.0, bcols["l3"][:].to_broadcast([P, BG]),
                op0=OP.mult, op1=OP.add)
            yield
            z3 = shared.tile([P, GT, HEADS], F32, tag="z3")
            st["z3"] = z3
            for t in range(GT):
                pt = pst.tile([P, 4 * P], F32, tag="pst")
                nc.tensor.transpose(pt[:, 0:P], z3T[:, t * P:(t + 1) * P],
                                    ident[:])
                nc.vector.tensor_copy(z3[:, t, :], pt[:, 0:P])
            yield

        # ---------------- phase B3: kwta3, x3T, l4, out ----------------------
        def phase_b3(g, st):
            col0 = g * BG
            x3 = big.tile([P, GT, HEADS], F32, tag="x3")
            yield from kwta(st["z3"], x3, st["kk2"], HEADS)
            x3T = shared.tile([P, 1, BG], F32, tag="hx3", name="x3T")
            yield from transpose_x(x3, x3T, HEADS)
            ps4 = psa.tile([P, BG], F32, tag="psa")
            wa = wslab_a("l4", 0, 1, 0, P)
            nc.tensor.matmul(ps4[:], wa[:, 0, :], x3T[:, 0, :],
                             start=True, stop=True)
            og = big.tile([P, BG], F32, tag="zot", name="og")
            nc.vector.scalar_tensor_tensor(
                og[:], ps4[:], 1.0, bcols["l4"][:].to_broadcast([P, BG]),
                op0=OP.mult, op1=OP.add)
            nc.sync.dma_start(outT[:, col0:col0 + BG], og[:])
            yield

        # ---------------- weave ------------------------------------------
        sts = [dict() for _ in range(NG)]

        def weave(gens):
            active = list(gens)
            while active:
                keep = []
                for it in active:
                    try:
                        next(it)
                        keep.append(it)
                    except StopIteration:
                        pass
                active = keep

        def phase_a(g, st):
            yield from phase_a1(g, st)
            yield from phase_a2(g, st)

        def seq(*gens):
            for gi in gens:
                yield from gi

        slots = [
            [phase_a(0, sts[0])],
            [phase_a(1, sts[1])],
            [phase_a(2, sts[2]), phase_b1(0, sts[0])],
            [phase_a(3, sts[3]), phase_b2(0, sts[0]), phase_b1(1, sts[1])],
            [phase_b3(0, sts[0]), phase_b2(1, sts[1]),
             seq(phase_b1(2, sts[2]), phase_b1(3, sts[3]))],
            [phase_b3(1, sts[1]),
             seq(phase_b2(2, sts[2]), phase_b2(3, sts[3]))],
            [seq(phase_b3(2, sts[2]), phase_b3(3, sts[3]))],
        ]
        for s in slots:
            weave(s)


# ----------------------------------------------------------------------------
# host wrapper
# ----------------------------------------------------------------------------

_CACHE = {}


def _get_program():
    if "nc" not in _CACHE:
        _CACHE["nc"] = build_program()
    return _CACHE["nc"]


def _fingerprint(arrs):
    out = []
    for a in arrs:
        out.append((id(a), a.shape, a.dtype.str,
                    float(a.flat[0]), float(a.flat[-1])))
    return tuple(out)


def _prep_weights(ws):
    """ws: dict name -> (w, b). Returns the replicated input map (cached)."""
    arrs = [a for pair in ws.values() for a in pair]
    key = _fingerprint(arrs)
    hit = _CACHE.get("wkey")
    if hit == key:
        return _CACHE["wmap"]
    m = {}
    shards = {}
    for name, (w, b) in ws.items():
        w = np.asarray(w, dtype=np.float32)
        b = np.asarray(b, dtype=np.float32)
        if name in IN_LAYERS:
            wT = np.ascontiguousarray(w[:, :1024].T)
            m[f"{name}_tail"] = np.ascontiguousarray(
                np.vstack([w[:, 1024:1028].T, b[None, :]]))
        else:
            wT = np.ascontiguousarray(w.T)
            if name in ("l3", "l4"):
                m[f"{name}_bcol"] = np.ascontiguousarray(
                    np.broadcast_to(b[:, None], (P, 1)))
            else:
                m[f"_b_{name}"] = b
        shards[name] = wT
    m["brows"] = np.concatenate(
        [m.pop(f"_b_{n}") for n in ("cx12", "cx22", "cx32", "l2")])[None, :]
    wflat = np.concatenate([shards[n].reshape(-1) for n in W_ORDER])
    wsh = [wflat[c * WSH:(c + 1) * WSH] for c in range(NCORES)]
    _CACHE["wkey"] = key
    _CACHE["wmap"] = (m, {"wflat_sh": wsh})
    return m, {"wflat_sh": wsh}


def kernel(**inputs):
    _trace = bool(inputs.pop("_trace", False))
    nc = _get_program()
    state = np.asarray(inputs["state"], dtype=np.float32)
    task = np.asarray(inputs["task_indicator"], dtype=np.float32)
    ws = {n: (inputs[f"{n}_w"], inputs[f"{n}_b"])
          for n in list(IN_LAYERS) + list(HID_LAYERS)}
    common, shards = _prep_weights(ws)
    in_maps = []
    for c in range(NCORES):
        m = dict(common)
        m["state"] = state[c * BC:(c + 1) * BC]
        m["task"] = task[c * BC:(c + 1) * BC]
        for sk, sv in shards.items():
            m[sk] = sv[c]
        in_maps.append(m)
    res = run_bass_kernel_spmd(nc, in_maps, core_ids=list(range(NCORES)),
                               trace=_trace)
    kernel.last_exec_time_ns = res.exec_time_ns
    out = np.concatenate([r["outT"].T for r in res.results], axis=0)
    return np.ascontiguousarray(out, dtype=np.float32)


kernel.last_exec_time_ns = None


# revision 33
# speedup vs baseline: 1.2423x; 1.0854x over previous
"""Trainium2 Bass kernel for nn_NeuralNetwork_S (kwta / topk_masking) — v3.

Key design points (vs the original 3-term f32r-split baseline):
- Native fp32 matmuls (HW probe: max rel err 1.8e-7, identical to the
  3-term 12-bit f32r split) -> no hi/lo splits anywhere: half the shipped
  bytes, no host-side rne12, no DVE subtract passes. (bf16/fp16 split
  variants fail the 2e-2 gate: kwta/argmax near-ties amplify operand
  rounding; measured on CPU.)
- Weights ship SHARDED 1/8 per core (18.8MB total instead of 162MB
  replicated) and are AllGather'd on-device into a DRAM bounce buffer as
  one flat 18.8MB collective; per-layer views are rearranged APs into it.
- Host ships raw per-core row slices of state/task (zero-copy views) and
  flat weight-shard views; w^T prep is cached across calls keyed on a
  fingerprint of the weight arrays. ci transpose happens on device
  (PE transpose + DVE copy; GPSIMD cannot read PSUM).
- Biases of the 4 IN-facing layers fold into an augmented K=5 tail matmul
  (task^T rows + ones row) x (w_tail rows + bias row) — free on PE since
  matmul cost is N-driven.
- Software-pipelined emission via woven generators: per group g, phase A
  (ci transpose + l1 + cx chains) and phases B1/B2/B3 (kwta1+l2 / kwta2+l3
  / kwta3+l4) interleave so group g's kwta bisections (ACT/Pool/DVE) hide
  under group g+1's matmul stream (PE stays >80% busy; sim ~1.69ms).
- kwta bisection: per-row dynamic-k threshold found by 12-iter bisection;
  counts on ACT only (sigmoid step w/ 2^100 scale + accum is exact;
  DVE tensor_tensor_reduce and Pool accum_out both break on real HW),
  interval ping-pong arithmetic on Pool, selects on DVE — sized to the
  engines' 4-deep wait stations to avoid queue-head deadlocks.
"""

import sys

_TRN = "/opt/trn_rl_repo"
if _TRN not in sys.path:
    sys.path.insert(0, _TRN)

import numpy as np
import concourse.bass as bass
import concourse.mybir as mybir
import concourse.tile as tile
from concourse import bacc
from concourse.bass_utils import run_bass_kernel_spmd
from concourse.masks import make_identity

P = 128
B = 16384
NCORES = 8
BC = B // NCORES          # 2048 rows per core
BG = 512                  # rows per group
NG = BC // BG             # 4 groups
GT = BG // P              # 4 row-tiles per group
IN = 1028
KIN = 8                   # full 128-row k-chunks of the 1024 state features
HID = 1024
HID2 = 512
HEADS = 128

F32 = mybir.dt.float32
U8 = mybir.dt.uint8
I32 = mybir.dt.int32
U32 = mybir.dt.uint32
BF16 = mybir.dt.bfloat16
AF = mybir.ActivationFunctionType
OP = mybir.AluOpType
AX = mybir.AxisListType

SCALE = float(2.0 ** 100)
ITERS = {1024: 12, 512: 12, 128: 10}
THIRD = 1.0 / 3.0

# layer tables ---------------------------------------------------------------
# IN-layers (read ci): (name, out, form); form 'a' = out-on-partitions,
# 'b' = rows-on-partitions
IN_LAYERS = {"cx11": HID, "cx21": HID2, "cx31": HEADS, "l1": HID}
# hidden layers: name -> (k_in, out)
HID_LAYERS = {"cx12": (HID, HID), "cx22": (HID2, HID2), "cx32": (HEADS, HEADS),
              "l2": (HID, HID2), "l3": (HID2, HEADS), "l4": (HEADS, HEADS)}
W_DIMS = {"l1": (1024, HID), "cx11": (1024, HID), "cx12": (HID, HID),
          "cx21": (1024, HID2), "cx22": (HID2, HID2), "cx31": (1024, HEADS),
          "cx32": (HEADS, HEADS), "l2": (HID, HID2), "l3": (HID2, HEADS),
          "l4": (HEADS, HEADS)}
W_ORDER = ["l1", "cx11", "cx12", "cx21", "cx22", "cx31", "cx32",
           "l2", "l3", "l4"]
W_OFF = {}
_off = 0
for _n in W_ORDER:
    W_OFF[_n] = _off
    _off += W_DIMS[_n][0] * W_DIMS[_n][1]
WTOT = _off          # 4,685,824 floats
WSH = WTOT // NCORES


def build_program():
    nc = bacc.Bacc("TRN2", target_bir_lowering=False, debug=False)
    d = {}

    def din(name, shape, dt=F32):
        d[name] = nc.dram_tensor(name, list(shape), dt, kind="ExternalInput")
        return d[name]

    din("state", [BC, 1024])
    din("task", [BC, 4])
    for name, o in IN_LAYERS.items():
        din(f"{name}_tail", [5, o])
    din("wflat_sh", [WSH])
    din("brows", [1, 2176])   # cx12(1024) | cx22(512) | cx32(128) | l2(512)
    for name in ("l3", "l4"):
        din(f"{name}_bcol", [P, 1])

    outT = nc.dram_tensor("outT", [P, BC], F32, kind="ExternalOutput")

    with tile.TileContext(nc) as tc:
        _emit(tc, nc, d, outT)
    nc.compile()
    return nc


def _emit(tc, nc, d, outT):
    import contextlib

    ctx = contextlib.ExitStack()
    with ctx:
        big = ctx.enter_context(tc.tile_pool(name="big", bufs=1))
        dbuf = ctx.enter_context(tc.tile_pool(name="dbuf", bufs=1))
        shared = ctx.enter_context(tc.tile_pool(name="shared", bufs=2))
        wts = ctx.enter_context(tc.tile_pool(name="wts", bufs=2))
        cons = ctx.enter_context(tc.tile_pool(name="cons", bufs=1))
        small = ctx.enter_context(tc.tile_pool(name="small", bufs=4))
        scr = ctx.enter_context(tc.tile_pool(name="scr", bufs=1))
        dram = ctx.enter_context(tc.tile_pool(name="dram", bufs=1,
                                               space="DRAM"))
        psb = ctx.enter_context(tc.tile_pool(name="psb", bufs=1, space="PSUM"))
        psa = ctx.enter_context(tc.tile_pool(name="psa", bufs=2, space="PSUM"))
        pst = ctx.enter_context(tc.tile_pool(name="pst", bufs=2, space="PSUM"))

        # constants ----------------------------------------------------------
        ident = cons.tile([P, P], F32, tag="ident")
        make_identity(nc, ident[:])
        negbig = cons.tile([P, 1], F32, tag="negbig")
        nc.vector.memset(negbig[:], -1.0e30)
        iota8 = cons.tile([P, 8], F32, tag="iota8")
        iota8u = small.tile([P, 8], U32, tag="iota8u")
        nc.gpsimd.iota(iota8u[:], pattern=[[1, 8]], base=0, channel_multiplier=0)
        nc.vector.tensor_copy(iota8[:], iota8u[:])
        zbias = cons.tile([P, 1], F32, tag="zbias")
        nc.vector.memset(zbias[:], 0.0)

        # resident weights: tails + breps + bcols -----------------------------
        tails = {}
        for name, o in IN_LAYERS.items():
            t = cons.tile([5, o], F32, tag=f"tail_{name}")
            nc.sync.dma_start(t[:], d[f"{name}_tail"][:])
            tails[name] = t
        brow = shared.tile([1, 2176], F32, tag="big16", name="brow")
        nc.sync.dma_start(brow[:], d["brows"][:])
        ones1 = small.tile([1, P], F32, tag="ones1")
        nc.vector.memset(ones1[:], 1.0)
        breps = {}
        _boff = 0
        for name in ("cx12", "cx22", "cx32", "l2"):
            o = HID_LAYERS[name][1]
            t = cons.tile([P, o], F32, tag=f"brep_{name}")
            for c0 in range(0, o, 512):
                cw = min(512, o - c0)
                psB = psa.tile([P, BG], F32, tag="psa", name="psB")
                nc.tensor.matmul(psB[:, 0:cw], ones1[0:1, :],
                                 brow[0:1, _boff + c0:_boff + c0 + cw],
                                 start=True, stop=True)
                nc.vector.tensor_copy(t[:, c0:c0 + cw], psB[:, 0:cw])
            breps[name] = t
            _boff += o
        bcols = {}
        for name in ("l3", "l4"):
            t = cons.tile([P, 1], F32, tag=f"bcol_{name}")
            nc.sync.dma_start(t[:], d[f"{name}_bcol"][:])
            bcols[name] = t

        state_r = d["state"].rearrange("(n p) f -> p n f", p=P)   # [P,16,1024]
        task_r = d["task"].rearrange("(n p) f -> p n f", p=P)     # [P,16,4]

        # ---- weight all-gather: one flat 18.8MB gather (BW ramps with
        # size; 15us fixed overhead per collective favors a single one).
        gin = dram.tile([WSH], F32, tag="gin")
        gout = nc.dram_tensor("wflat_gout", [WTOT], F32, kind="Internal",
                              addr_space="Shared")
        nc.sync.dma_start(gin[:], d["wflat_sh"][:])
        nc.gpsimd.collective_compute(
            "AllGather", mybir.AluOpType.bypass,
            replica_groups=[list(range(NCORES))],
            ins=[gin.opt()], outs=[gout[:]])
        gathered = {}
        for name in W_ORDER:
            k, o = W_DIMS[name]
            off = W_OFF[name]
            gathered[name] = gout[off:off + k * o].rearrange(
                "(c p o) -> p c o", p=P, o=o)

        def wslab_b(name, k, n0, nw):
            """(b)-form moving slab [P, 1, nw] from wT rows [k*128, +128)."""
            t = wts.tile([P, 1, nw], F32, tag="wb")
            nc.sync.dma_start(t[:], gathered[name][:, k:k + 1, n0:n0 + nw])
            return t

        def wslab_a(name, k0, kc, m0, mw):
            """(a)-form stationary slab [P, kc<=4, mw] (k-chunks k0..k0+kc)."""
            t = wts.tile([P, kc, mw], F32, tag="wa")
            nc.sync.dma_start(t[:], gathered[name][:, k0:k0 + kc, m0:m0 + mw])
            return t

        # ---------------- phase A1: ci transpose, l1, cx1 chain --------------
        def phase_a1(g, st):
            col0 = g * BG
            ciT = shared.tile([P, KIN, BG], F32, tag="big16", name="ciT")
            st["ciT"] = ciT
            taskT = big.tile([5, BG], F32, tag="taskT")
            st["taskT"] = taskT
            tTASK = small.tile([P, GT, 5], F32, tag="tTASK")
            nc.sync.dma_start(tTASK[:, :, 0:4], task_r[:, g * GT:(g + 1) * GT, :])
            nc.gpsimd.memset(tTASK[:, :, 4:5], 1.0)
            yield
            # transpose ci into [feature-part, row] layout
            for t in range(GT):
                sROW = dbuf.tile([P, 1024], F32, tag="sROW")
                nc.sync.dma_start(sROW[:], state_r[:, g * GT + t, :])
                for c0 in (0, 4):
                    ps = pst.tile([P, 4 * P], F32, tag="pst")
                    for c in range(c0, c0 + 4):
                        nc.tensor.transpose(
                            ps[:, (c - c0) * P:(c - c0 + 1) * P],
                            sROW[:, c * P:(c + 1) * P], ident[:])
                    dst = ciT[:, c0:c0 + 4, t * P:(t + 1) * P]
                    src = ps[:].rearrange("p (c q) -> p c q", q=P)
                    nc.vector.tensor_copy(dst, src)
                    yield
                pt = pst.tile([P, 4 * P], F32, tag="pst")
                nc.tensor.transpose(pt[0:5, 0:P], tTASK[:, t, :], ident[:])
                nc.vector.tensor_copy(taskT[0:5, t * P:(t + 1) * P],
                                      pt[0:5, 0:P])
                yield

            # ---- l1 (b): z1 [P, GT, 1024]
            z1 = shared.tile([P, GT, HID], F32, tag="z1", name="z1")
            st["z1"] = z1
            for n0 in range(0, HID, 512):
                ps = psb.tile([P, GT, 512], F32, tag="psb")
                for k in range(KIN):
                    wb = wslab_b("l1", k, n0, 512)
                    for t in range(GT):
                        nc.tensor.matmul(
                            ps[:, t, :], ciT[:, k, t * P:(t + 1) * P],
                            wb[:, 0, :], start=(k == 0), stop=False)
                    yield
                for t in range(GT):
                    nc.tensor.matmul(
                        ps[:, t, :], taskT[0:5, t * P:(t + 1) * P],
                        tails["l1"][0:5, n0:n0 + 512], start=False, stop=True)
                yield
                for t in range(GT):
                    nc.vector.tensor_copy(z1[:, t, n0:n0 + 512], ps[:, t, :])
                yield

            # ---- cx1 chain -> kk0
            yield from cx_chain(g, st, 0)

        # ---------------- phase A2: cx2/cx3 chains ---------------------------
        def phase_a2(g, st):
            yield from cx_chain(g, st, 1)
            yield from cx_chain(g, st, 2)

        CX_DEFS = [("cx11", "cx12", HID, 8), ("cx21", "cx22", HID2, 4),
                   ("cx31", "cx32", HEADS, 1)]

        def cx_chain(g, st, cn):
            ciT = st["ciT"]
            taskT = st["taskT"]
            if True:
                pre, post, hidn, mch = CX_DEFS[cn]
                kc_pre = KIN
                httag = {0: "hx1", 1: "hx2", 2: "hx3"}[cn]
                hT = shared.tile([P, mch, BG], F32, tag=httag, name=f"hT{cn}")
                for m in range(mch):
                    ps = psa.tile([P, BG], F32, tag="psa")
                    for k0 in range(0, kc_pre, 4):
                        wa = wslab_a(pre, k0, 4, m * P, P)
                        for k in range(k0, k0 + 4):
                            nc.tensor.matmul(ps[:], wa[:, k - k0, :],
                                             ciT[:, k, :],
                                             start=(k == 0), stop=False)
                    nc.tensor.matmul(ps[:], tails[pre][0:5, m * P:(m + 1) * P],
                                     taskT[0:5, :], start=False, stop=True)
                    nc.scalar.activation(hT[:, m, :], ps[:], AF.Tanh,
                                         bias=zbias[:], scale=1.0)
                    yield

                # second layer (b) + incremental argmax
                kk = small.tile([P, GT], F32, tag=f"kk{cn}", name="kk")
                st[f"kk{cn}"] = kk
                kin2, out2 = HID_LAYERS[post]
                bestm = small.tile([P, GT], F32, tag="bestm")
                kkA = small.tile([P, GT], F32, tag="kkA")
                n0s = list(range(0, out2, 512))
                for ci_, n0 in enumerate(n0s):
                    nw = min(512, out2)
                    ps = psb.tile([P, GT, 512], F32, tag="psb")
                    for k in range(mch):
                        wb = wslab_b(post, k, n0, nw)
                        for t in range(GT):
                            nc.tensor.matmul(
                                ps[:, t, 0:nw], hT[:, k, t * P:(t + 1) * P],
                                wb[:, 0, :], start=(k == 0), stop=(k == mch - 1))
                        yield
                    m8 = small.tile([P, 8], F32, tag="am8")
                    idx = small.tile([P, 8], U32, tag="aidx")
                    idxf = small.tile([P, 8], F32, tag="aidxf")
                    for t in range(GT):
                        zcx = big.tile([P, 512], F32, tag="zcx", name="zcx")
                        nc.vector.scalar_tensor_tensor(
                            zcx[:, 0:nw], ps[:, t, 0:nw], 1.0,
                            breps[post][:, n0:n0 + nw], op0=OP.mult, op1=OP.add)
                        nc.vector.max(out=m8[:], in_=zcx[:, 0:nw])
                        nc.vector.max_index(idx[:], m8[:], zcx[:, 0:nw])
                        nc.vector.tensor_copy(idxf[:, 0:1], idx[:, 0:1])
                        if ci_ == 0 and len(n0s) == 1:
                            nc.vector.tensor_copy(kk[:, t:t + 1], idxf[:, 0:1])
                        elif ci_ == 0:
                            nc.vector.tensor_copy(kkA[:, t:t + 1], idxf[:, 0:1])
                            nc.vector.tensor_copy(bestm[:, t:t + 1],
                                                  m8[:, 0:1])
                        else:
                            gtu = small.tile([P, 1], U8, tag="agt")
                            nc.vector.tensor_tensor(
                                gtu[:], m8[:, 0:1], bestm[:, t:t + 1],
                                op=OP.is_gt)
                            i2 = small.tile([P, 1], F32, tag="ai2")
                            nc.vector.tensor_scalar(
                                i2[:], idxf[:, 0:1], float(n0), None,
                                op0=OP.add)
                            nc.vector.select(kk[:, t:t + 1], gtu[:], i2[:],
                                             kkA[:, t:t + 1])
                        yield

        # ---------------- kwta bisection ------------------------------------
        def kwta(zg, xg, kk, n):
            I = ITERS[n]
            loA = small.tile([P, GT], F32, tag="kwloA")
            loB = small.tile([P, GT], F32, tag="kwloB")
            hiA = small.tile([P, GT], F32, tag="kwhiA")
            hiB = small.tile([P, GT], F32, tag="kwhiB")
            chA = small.tile([P, GT], F32, tag="kwchA")
            chB = small.tile([P, GT], F32, tag="kwchB")
            cnt = small.tile([P, GT], F32, tag="kwcnt")
            kp1 = small.tile([P, GT], F32, tag="kwkp1")
            msum = small.tile([P, GT], F32, tag="kwmsum")
            mid = small.tile([P, GT], F32, tag="kwmid")
            nbias = small.tile([P, GT], F32, tag="kwnb")
            mn = small.tile([P, GT], F32, tag="kwmn")
            selu = small.tile([P, GT], U8, tag="kwselu")
            trash = scr.tile([P, n], BF16, tag=f"kwA{n}", name="trash")

            nc.gpsimd.tensor_scalar(kp1[:], kk[:], 1.0, None, op0=OP.add)
            nc.gpsimd.memset(chA[:], 0.0)
            for t in range(GT):
                nc.vector.reduce_max(hiA[:, t:t + 1], zg[:, t, :], axis=AX.X)
                nc.vector.tensor_reduce(out=mn[:, t:t + 1], in_=zg[:, t, :],
                                        op=OP.min, axis=AX.X)
            nc.gpsimd.tensor_scalar(loA[:], mn[:], 1.0, None, op0=OP.subtract)
            yield

            lo, hi, ch = loA, hiA, chA
            lon, hin, chn = loB, hiB, chB
            for it in range(I):
                nc.gpsimd.tensor_tensor(msum[:], lo[:], hi[:], op=OP.add)
                nc.gpsimd.tensor_scalar(mid[:], msum[:], 0.5, None,
                                        op0=OP.mult)
                nc.gpsimd.tensor_scalar(nbias[:], mid[:], -SCALE, None,
                                        op0=OP.mult)
                for t in range(GT):
                    nc.scalar.activation(
                        trash[:], zg[:, t, :], AF.Sigmoid,
                        bias=nbias[:, t:t + 1], scale=SCALE,
                        accum_out=cnt[:, t:t + 1])
                nc.vector.tensor_tensor(selu[:], cnt[:], kp1[:], op=OP.is_ge)
                nc.vector.select(lon[:], selu[:], mid[:], lo[:])
                nc.vector.select(hin[:], selu[:], hi[:], mid[:])
                nc.vector.select(chn[:], selu[:], ch[:], cnt[:])
                lo, lon = lon, lo
                hi, hin = hin, hi
                ch, chn = chn, ch
                yield

            chii = small.tile([P, GT], I32, tag="kwchii")
            nc.vector.tensor_scalar(chn[:], ch[:], 0.25, None, op0=OP.subtract)
            nc.vector.tensor_copy(chii[:], chn[:])
            nc.vector.tensor_copy(ch[:], chii[:])
            rm1 = small.tile([P, GT], F32, tag="kwrm1")
            nc.vector.tensor_tensor(rm1[:], kk[:], ch[:], op=OP.subtract)
            yield

            for t in range(GT):
                m1 = scr.tile([P, n], F32, tag=f"kwA{n}", name="m1")
                gu8 = scr.tile([P, n], U8, tag=f"kwgu{n}", name="gu8")
                msk = scr.tile([P, n], F32, tag=f"kwmsk{n}", name="msk")
                nc.gpsimd.tensor_scalar(m1[:], zg[:, t, :], lo[:, t:t + 1],
                                        None, op0=OP.max)
                nc.vector.tensor_scalar(gu8[:], zg[:, t, :], hi[:, t:t + 1],
                                        None, op0=OP.is_gt)
                nc.vector.select(msk[:], gu8[:], negbig[:].to_broadcast([P, n]),
                                 m1[:])
                m8 = small.tile([P, 8], F32, tag="kwm8")
                nc.vector.max(out=m8[:], in_=msk[:])
                eq = small.tile([P, 8], F32, tag="kweq")
                nc.vector.tensor_scalar(eq[:], iota8[:], rm1[:, t:t + 1],
                                        None, op0=OP.is_equal)
                pr = small.tile([P, 8], F32, tag="kwpr")
                nc.vector.tensor_tensor(pr[:], eq[:], m8[:], op=OP.mult)
                u = small.tile([P, 1], F32, tag="kwu")
                nc.vector.reduce_sum(u[:], pr[:], axis=AX.X)
                yield
                geu = scr.tile([P, n], U8, tag=f"kwgu{n}", name="geu")
                nc.vector.tensor_scalar(geu[:], zg[:, t, :], u[:], None,
                                        op0=OP.is_gt)
                zth = scr.tile([P, n], F32, tag=f"kwA{n}", name="zth")
                nc.gpsimd.tensor_scalar(zth[:], zg[:, t, :], THIRD, None,
                                        op0=OP.mult)
                nc.vector.select(xg[:, t, :], geu[:], zg[:, t, :], zth[:])
                yield

        # transpose [P, GT, n] -> xT [P, n//P, BG]
        def transpose_x(xg, xT, n):
            nch = n // P
            for t in range(GT):
                for c0 in range(0, nch, 4):
                    cw = min(4, nch - c0)
                    ps = pst.tile([P, 4 * P], F32, tag="pst")
                    for c in range(c0, c0 + cw):
                        nc.tensor.transpose(
                            ps[:, (c - c0) * P:(c - c0 + 1) * P],
                            xg[:, t, c * P:(c + 1) * P], ident[:])
                    dst = xT[:, c0:c0 + cw, t * P:(t + 1) * P]
                    src = ps[:, 0:cw * P].rearrange("p (c q) -> p c q", q=P)
                    nc.vector.tensor_copy(dst, src)
                    yield

        # ---------------- phase B1: kwta1, x1T, l2 ---------------------------
        def phase_b1(g, st):
            x1 = shared.tile([P, GT, HID], F32, tag="big16", name="x1")
            yield from kwta(st["z1"], x1, st["kk0"], HID)
            x1T = shared.tile([P, HID // P, BG], F32, tag="hx1", name="x1T")
            yield from transpose_x(x1, x1T, HID)
            z2 = shared.tile([P, GT, HID2], F32, tag="z2")
            st["z2"] = z2
            ps = psb.tile([P, GT, 512], F32, tag="psb")
            for k in range(HID // P):
                wb = wslab_b("l2", k, 0, HID2)
                for t in range(GT):
                    nc.tensor.matmul(
                        ps[:, t, :], x1T[:, k, t * P:(t + 1) * P],
                        wb[:, 0, :], start=(k == 0), stop=(k == HID // P - 1))
                yield
            for t in range(GT):
                nc.vector.scalar_tensor_tensor(
                    z2[:, t, :], ps[:, t, :], 1.0, breps["l2"][:],
                    op0=OP.mult, op1=OP.add)
            yield

        # ---------------- phase B2: kwta2, x2T, l3 ---------------------------
        def phase_b2(g, st):
            x2 = big.tile([P, GT, HID2], F32, tag="x2")
            yield from kwta(st["z2"], x2, st["kk1"], HID2)
            x2T = shared.tile([P, HID2 // P, BG], F32, tag="hx2", name="x2T")
            yield from transpose_x(x2, x2T, HID2)
            ps3 = psa.tile([P, BG], F32, tag="psa")
            wa = wslab_a("l3", 0, HID2 // P, 0, P)
            for k in range(HID2 // P):
                nc.tensor.matmul(ps3[:], wa[:, k, :], x2T[:, k, :],
                                 start=(k == 0), stop=(k == HID2 // P - 1))
            z3T = big.tile([P, BG], F32, tag="zot", name="z3T")
            nc.vector.scalar_tensor_tensor(
                z3T[:], ps3[:], 1.0, bcols["l3"][:].to_broadcast([P, BG]),
                op0=OP.mult, op1=OP.add)
            yield
            z3 = shared.tile([P, GT, HEADS], F32, tag="z3")
            st["z3"] = z3
            for t in range(GT):
                pt = pst.tile([P, 4 * P], F32, tag="pst")
                nc.tensor.transpose(pt[:, 0:P], z3T[:, t * P:(t + 1) * P],
                                    ident[:])
                nc.vector.tensor_copy(z3[:, t, :], pt[:, 0:P])
            yield

        # ---------------- phase B3: kwta3, x3T, l4, out ----------------------
        def phase_b3(g, st):
            col0 = g * BG
            x3 = big.tile([P, GT, HEADS], F32, tag="x3")
            yield from kwta(st["z3"], x3, st["kk2"], HEADS)
            x3T = shared.tile([P, 1, BG], F32, tag="hx3", name="x3T")
            yield from transpose_x(x3, x3T, HEADS)
            ps4 = psa.tile([P, BG], F32, tag="psa")
            wa = wslab_a("l4", 0, 1, 0, P)
            nc.tensor.matmul(ps4[:], wa[:, 0, :], x3T[:, 0, :],
                             start=True, stop=True)
            og = big.tile([P, BG], F32, tag="zot", name="og")
            nc.vector.scalar_tensor_tensor(
                og[:], ps4[:], 1.0, bcols["l4"][:].to_broadcast([P, BG]),
                op0=OP.mult, op1=OP.add)
            nc.sync.dma_start(outT[:, col0:col0 + BG], og[:])
            yield

        # ---------------- weave ------------------------------------------
        sts = [dict() for _ in range(NG)]

        def weave(gens):
            active = list(gens)
            while active:
                keep = []
                for it in active:
                    try:
                        next(it)
                        keep.append(it)
                    except StopIteration:
                        pass
                active = keep

        def phase_a(g, st):
            yield from phase_a1(g, st)
            yield from phase_a2(g, st)

        def seq(*gens):
            for gi in gens:
                yield from gi

        slots = [
            [phase_a(0, sts[0])],
            [phase_a(1, sts[1])],
            [phase_a(2, sts[2]), phase_b1(0, sts[0])],
            [phase_a(3, sts[3]), phase_b2(0, sts[0]), phase_b1(1, sts[1])],
            [phase_b3(0, sts[0]), phase_b2(1, sts[1]),
             seq(phase_b1(2, sts[2]), phase_b1(3, sts[3]))],
            [phase_b3(1, sts[1]),
             seq(phase_b2(2, sts[2]), phase_b2(3, sts[3]))],
            [seq(phase_b3(2, sts[2]), phase_b3(3, sts[3]))],
        ]
        for s in slots:
            weave(s)


# ----------------------------------------------------------------------------
# host wrapper
# ----------------------------------------------------------------------------

_CACHE = {}


def _get_program():
    if "nc" not in _CACHE:
        _CACHE["nc"] = build_program()
    return _CACHE["nc"]


def _fingerprint(arrs):
    out = []
    for a in arrs:
        out.append((id(a), a.shape, a.dtype.str,
                    float(a.flat[0]), float(a.flat[-1])))
    return tuple(out)


def _prep_weights(ws):
    """ws: dict name -> (w, b). Returns the replicated input map (cached)."""
    arrs = [a for pair in ws.values() for a in pair]
    key = _fingerprint(arrs)
    hit = _CACHE.get("wkey")
    if hit == key:
        return _CACHE["wmap"]
    m = {}
    shards = {}
    for name, (w, b) in ws.items():
        w = np.asarray(w, dtype=np.float32)
        b = np.asarray(b, dtype=np.float32)
        if name in IN_LAYERS:
            wT = np.ascontiguousarray(w[:, :1024].T)
            m[f"{name}_tail"] = np.ascontiguousarray(
                np.vstack([w[:, 1024:1028].T, b[None, :]]))
        else:
            wT = np.ascontiguousarray(w.T)
            if name in ("l3", "l4"):
                m[f"{name}_bcol"] = np.ascontiguousarray(
                    np.broadcast_to(b[:, None], (P, 1)))
            else:
                m[f"_b_{name}"] = b
        shards[name] = wT
    m["brows"] = np.concatenate(
        [m.pop(f"_b_{n}") for n in ("cx12", "cx22", "cx32", "l2")])[None, :]
    wflat = np.concatenate([shards[n].reshape(-1) for n in W_ORDER])
    wsh = [wflat[c * WSH:(c + 1) * WSH] for c in range(NCORES)]
    _CACHE["wkey"] = key
    _CACHE["wmap"] = (m, {"wflat_sh": wsh})
    return m, {"wflat_sh": wsh}


def kernel(**inputs):
    _trace = bool(inputs.pop("_trace", False))
    nc = _get_program()
    state = np.asarray(inputs["state"], dtype=np.float32)
    task = np.asarray(inputs["task_indicator"], dtype=np.float32)
    ws = {n: (inputs[f"{n}_w"], inputs[f"{n}_b"])
          for n in list(IN_LAYERS) + list(HID_LAYERS)}
    common, shards = _prep_weights(ws)
    in_maps = []
    for c in range(NCORES):
        m = dict(common)
        m["state"] = state[c * BC:(c + 1) * BC]
        m["task"] = task[c * BC:(c + 1) * BC]
        for sk, sv in shards.items():
            m[sk] = sv[c]
        in_maps.append(m)
    res = run_bass_kernel_spmd(nc, in_maps, core_ids=list(range(NCORES)),
                               trace=_trace)
    kernel.last_exec_time_ns = res.exec_time_ns
    out = np.concatenate([r["outT"].T for r in res.results], axis=0)
    return np.ascontiguousarray(out, dtype=np.float32)


kernel.last_exec_time_ns = None
